# revision 16
# baseline (speedup 1.0000x reference)
"""Cross-attention kernel for Trainium2 (8 NeuronCores, SPMD data-parallel).

Problem: O = softmax(Q @ K^T) @ V with B=4, Lq=Lk=4096, D=64, fp32 (no
1/sqrt(d) scaling).

Sharding: 8 cores = 4 batches x 2 Lq-halves. Each core handles a
[2048, 64] Q shard against the full [4096, 64] K/V of its batch.
Independent outputs -> no collectives.

Score matmuls use fp8e4 (e4m3) in DoubleRow perf mode (1 out column
per 2.4GHz cycle with 256-deep contraction = 2x the bf16 MAC rate).
Precision is recovered with a 2-level split: Q = Qh + Ql, K = Kh + Kl
(Qh = e4m3(Q), Ql = e4m3(Q - Qh)); one DoubleRow matmul computes all
four cross terms at once:
  out = sum_i lhsT[:,i,:].T @ rhs[:,i,:]   (i = 0,1; contraction 128)
with partition rows 0:64 = d-index "high" terms, 64:128 = "low" terms:
  lhsT[:,0] = [Kh; Kl], lhsT[:,1] = [Kl; Kh] (k on free dim)
  rhs[:,0] = rhs[:,1] = [Qh; Ql]            (q on free dim, duplicated)
=> out = (Kh+Kl) stacked-contraction (Qh+Ql) = Q @ K^T to ~2^-8 rel.
Host-measured end-to-end rel_l2 vs the fp32 reference: ~5.9e-3.

Per-core pipeline (one unit = one k-chunk of 128 keys x 1024 q):
  - ST[k, q]: 4 DoubleRow matmuls of 256 out-cols each (~109ns) into
    two independent single-bank PSUM tiles sta/stb [128, 512] (each
    bufs=3) so the scalar-exp chain (sta) and vector-exp chain (stb)
    decouple and the exp->PSUM-reuse chain never stalls the PE. s=0/2
    start=True marks the 2KB bank pending-zero; s=1/3 start=False
    first-touch writes fresh.
  - P = exp(ST) -> bf16 SBUF: scalar engine does exact table exp on
    sta (q-cols 0:512), vector engine does a Schraudolph bit-trick exp
    (int16(A*s + B) reinterpreted as bf16 ~= e^s) on stb (512:1024).
    pt_a/pt_b come from separate pools (bufs=4 each) so the exp ->
    PV(u) -> pool-reuse chain has 3 units of slack (a shared pool
    halves that and was the main pipeline stall in earlier revisions).
  - Transposed PV, q on output partitions: for each 128-q sub-block j,
    out[q, 0:65] += matmul(lhsT=PT[:, j*128:(j+1)*128], rhs=VA chunk
    [128, 65]) -- 65 bf16 columns per matmul (~30ns back-to-back). VA
    = concat([V, ones], 1): col 64 accumulates the softmax denominator.
  - Each OT PSUM bank ([128, 4*65] = 4 j-groups) starts once, stops
    once. OT banks are pooled per qh (qh=1 reuses qh=0's two banks
    after the explicit mid-loop drain at u=33), which is what frees
    the 2 banks for the deeper sta/stb rotation.
  - 14 junk DoubleRow matmuls run during the input-DMA wait to ramp
    the PE HAM p-state (1.2 -> 2.4 GHz takes a few us of activity) so
    the real loop starts close to warm.
  - Steady state (~765ns/unit) is paced by exp-engine throughput (ACT
    674ns + DVE 689ns per unit, both ~90% busy). Fusing each unit's
    exp into one 1024-col instruction on alternating engines was tried
    and is ~10us WORSE (the PV/ST chains serialize behind the 1118ns
    instruction); the split 512/512 layout is the right shape.
  - Normalization (divide by col 64) happens on host after DMA-out,
    like the host-side transposes/fp8 packing.
"""

import sys

for _p in ("/opt/trn_rl_repo", "/opt/pypackages"):
    if _p not in sys.path:
        sys.path.insert(0, _p)

from contextlib import ExitStack

import ml_dtypes
import numpy as np

import concourse.bacc as bacc
import concourse.mybir as mybir
import concourse.tile as tile
from concourse.bass_utils import run_bass_kernel_spmd

# Problem constants (hardcoded per contract).
B, LQ, LK, D = 4, 4096, 4096, 64
N_CORES = 8
LQ_SHARD = LQ * B // N_CORES  # 2048
KC = 128  # k-chunk (PV contraction tile)
NKC = LK // KC  # 32
QB = 1024  # q extent per unit
NQB = LQ_SHARD // QB  # 2
NU = NQB * NKC  # 64 units
SW = 256  # out columns per DoubleRow score matmul (rhs free = 512 = max)
HB = 512  # half-unit q extent (one PSUM bank / one exp engine)
NJ = QB // 128  # q sub-blocks per unit (8)
N_WARM = 14  # junk matmuls to ramp the PE p-state during the DMA wait

F32 = mybir.dt.float32
BF16 = mybir.dt.bfloat16
I16 = mybir.dt.int16
E4 = mybir.dt.float8e4

BF16NP = ml_dtypes.bfloat16
E4NP = ml_dtypes.float8_e4m3

# Schraudolph constants for bf16: int16(A*s + B) bits viewed as bf16 ~ e^s.
SCH_A = float(128.0 / np.log(2.0))  # 184.664...
SCH_C = 8.0  # sawtooth centering shift
SCH_B = 128.0 * 127.0 - SCH_C + 0.5  # +0.5: float->int16 cast truncates


def _build_program():
    nc = bacc.Bacc(
        "TRN2",
        target_bir_lowering=False,
        debug=False,
        num_devices=N_CORES,
    )
    # [p, qh, i, col]: p<64 -> Qh[qh*1024+col, p], p>=64 -> Ql; i duplicated.
    q_d = nc.declare_dram_parameter("QD", [KC, NQB, 2, QB], E4, isOutput=False)
    k_d = nc.declare_dram_parameter("KD", [KC, NKC, 2, KC], E4, isOutput=False)
    va_d = nc.declare_dram_parameter("VA", [KC, NKC, D + 1], BF16, isOutput=False)
    # O[p, j*65 + d]: q sub-block j (q = j*128 + p), d in 0:64 out, 64 = den.
    o_d = nc.declare_dram_parameter("O", [KC, 2 * NJ * (D + 1)], F32, isOutput=True)

    with tile.TileContext(nc) as tc, ExitStack() as ctx:
        singles = ctx.enter_context(tc.tile_pool(name="singles", bufs=1))
        sta_pool = ctx.enter_context(tc.tile_pool(name="sta", bufs=3, space="PSUM"))
        stb_pool = ctx.enter_context(tc.tile_pool(name="stb", bufs=3, space="PSUM"))
        ot_pool = ctx.enter_context(tc.tile_pool(name="ot", bufs=1, space="PSUM"))
        pta_pool = ctx.enter_context(tc.tile_pool(name="pta", bufs=4))
        ptb_pool = ctx.enter_context(tc.tile_pool(name="ptb", bufs=4))
        ob_pool = ctx.enter_context(tc.tile_pool(name="ob", bufs=1))

        # Preload the exp activation table while input DMAs run.
        warm = singles.tile([1, 2], F32)
        nc.vector.memset(warm[:, :], 0.0)
        nc.scalar.activation(
            out=warm[:, :], in_=warm[:, :],
            func=mybir.ActivationFunctionType.Exp,
        )

        qsb = singles.tile([KC, NQB, 2, QB], E4, name="qsb")
        ksb = singles.tile([KC, NKC, 2, KC], E4, name="ksb")
        va = singles.tile([KC, NKC, D + 1], BF16, name="va")

        # Input DMAs, earliest-needed-first; small lead pieces so the
        # first score matmul is gated on ~320KB, not the full 2MB.
        nc.sync.dma_start(out=ksb[:, 0:2], in_=k_d[:, 0:2])
        nc.gpsimd.dma_start(out=qsb[:, 0], in_=q_d[:, 0])
        nc.sync.dma_start(out=va[:, 0:2, :], in_=va_d[:, 0:2, :])
        nc.sync.dma_start(out=ksb[:, 2:8], in_=k_d[:, 2:8])
        nc.sync.dma_start(out=va[:, 2:8, :], in_=va_d[:, 2:8, :])
        nc.sync.dma_start(out=ksb[:, 8:NKC], in_=k_d[:, 8:NKC])
        nc.sync.dma_start(out=va[:, 8:NKC, :], in_=va_d[:, 8:NKC, :])
        nc.gpsimd.dma_start(out=qsb[:, 1], in_=q_d[:, 1])

        # PE p-state warmup: junk DoubleRow matmuls on memset SBUF while
        # the input DMAs land (HAM ramps 1.2 -> 2.4 GHz only after a few
        # us of PE activity; these absorb most of that at otherwise-idle
        # time). They cycle the sta/stb pools ahead of the real loop.
        junk = singles.tile([KC, 2, SW], E4, name="junk")
        nc.gpsimd.memset(junk[:, :, :], 0.0)
        for w in range(N_WARM):
            if w % 2 == 0:
                wst = sta_pool.tile([KC, HB], F32, name=f"wst{w}", tag="sta")
            else:
                wst = stb_pool.tile([KC, HB], F32, name=f"wst{w}", tag="stb")
            nc.tensor.matmul(
                out=wst[:, 0:SW],
                lhsT=junk[:, :, 0:KC],
                rhs=junk[:, :, :],
                start=True,
                stop=True,
                perf_mode=mybir.MatmulPerfMode.DoubleRow,
                skip_group_check=True,
            )

        # One PSUM bank per OT tile (4 j-groups of 65 cols each), pooled
        # per qh: qh=1 reuses qh=0's two banks after the mid-loop drain,
        # freeing 2 banks for the deeper sta/stb rotation.
        ots = {}

        # Software-pipelined main loop: unit u = (qh, c) = (u // NKC, u % NKC).
        pts = [None] * NU

        def emit_scores_exp(u):
            qh, c = divmod(u, NKC)
            sta = sta_pool.tile([KC, HB], F32, tag="sta")
            stb = stb_pool.tile([KC, HB], F32, tag="stb")
            for s in range(4):
                tgt = sta if s < 2 else stb
                nc.tensor.matmul(
                    out=tgt[:, (s % 2) * SW : (s % 2 + 1) * SW],
                    lhsT=ksb[:, c, :, :],
                    rhs=qsb[:, qh, :, s * SW : (s + 1) * SW],
                    start=(s % 2 == 0),
                    stop=(s % 2 == 1),
                    perf_mode=mybir.MatmulPerfMode.DoubleRow,
                    skip_group_check=True,
                )
            pt_a = pta_pool.tile([KC, HB], BF16, tag="pta")
            pt_b = ptb_pool.tile([KC, HB], BF16, tag="ptb")
            nc.scalar.activation(
                out=pt_a[:, :],
                in_=sta[:, :],
                func=mybir.ActivationFunctionType.Exp,
            )
            nc.vector.tensor_scalar(
                pt_b[:, :].bitcast(I16),
                stb[:, :],
                SCH_A,
                SCH_B,
                mybir.AluOpType.mult,
                mybir.AluOpType.add,
            )
            pts[u] = (pt_a, pt_b)

        def emit_pv(u):
            qh, c = divmod(u, NKC)
            if c == 0:
                ots[qh] = [
                    ot_pool.tile(
                        [KC, 4 * (D + 1)], F32, name=f"ot{qh}{h}", tag=f"ot{h}"
                    )
                    for h in range(2)
                ]
            pt_a, pt_b = pts[u]
            for j in range(NJ):
                if j < 4:
                    lhsT = pt_a[:, j * KC : (j + 1) * KC]
                else:
                    lhsT = pt_b[:, (j - 4) * KC : (j - 3) * KC]
                h, g = j // 4, j % 4
                nc.tensor.matmul(
                    out=ots[qh][h][:, g * (D + 1) : (g + 1) * (D + 1)],
                    lhsT=lhsT,
                    rhs=va[:, c, :],
                    start=(c == 0 and g == 0),
                    stop=(c == NKC - 1 and g == 3),
                    skip_group_check=True,
                )

        # Output drain (normalization on host): PSUM->SBUF copies split
        # across the scalar and vector engines, then one DMA per qh
        # half. qh=0 drains mid-loop (its OT banks finish at u=31).
        W = 4 * (D + 1)
        ob = ob_pool.tile([KC, 4, W], F32, name="ob")

        def drain(qh):
            for h in range(2):
                t = qh * 2 + h
                if h == 0:
                    nc.scalar.activation(
                        out=ob[:, t, :], in_=ots[qh][h][:, :],
                        func=mybir.ActivationFunctionType.Copy,
                    )
                else:
                    nc.vector.tensor_copy(ob[:, t, :], ots[qh][h][:, :])
                eng = nc.sync if h == 0 else nc.gpsimd
                eng.dma_start(
                    out=o_d[:, t * W : (t + 1) * W],
                    in_=ob[:, t, :],
                )

        emit_scores_exp(0)
        emit_scores_exp(1)
        for u in range(2, NU):
            emit_scores_exp(u)
            emit_pv(u - 2)
            if u == 33:
                drain(0)
        emit_pv(NU - 2)
        emit_pv(NU - 1)
        drain(1)

    nc.finalize()
    return nc


_PROGRAM_CACHE = {}


def _get_program():
    if "nc" not in _PROGRAM_CACHE:
        _PROGRAM_CACHE["nc"] = _build_program()
    return _PROGRAM_CACHE["nc"]


def _make_in_maps(Q, K, V):
    Q = np.asarray(Q, dtype=np.float32)
    K = np.asarray(K, dtype=np.float32)
    V = np.asarray(V, dtype=np.float32)
    in_maps = []
    ones = np.ones((LK, 1), dtype=np.float32)
    for core in range(N_CORES):
        b, half = core // 2, core % 2
        q = Q[b, half * LQ_SHARD : (half + 1) * LQ_SHARD, :]  # [2048, 64]
        qh8 = q.astype(E4NP)
        ql8 = (q - qh8.astype(np.float32)).astype(E4NP)
        # qd[p, qh, i, col]: p<64 -> Qh[qh*1024+col, p]; p>=64 -> Ql.
        qd = np.empty((KC, NQB, 2, QB), dtype=E4NP)
        qhT = qh8.T.reshape(D, NQB, QB)
        qlT = ql8.T.reshape(D, NQB, QB)
        qd[0:D, :, 0, :] = qhT
        qd[0:D, :, 1, :] = qhT
        qd[D:KC, :, 0, :] = qlT
        qd[D:KC, :, 1, :] = qlT
        k = K[b]  # [4096, 64]
        kh8 = k.astype(E4NP)
        kl8 = (k - kh8.astype(np.float32)).astype(E4NP)
        khT = np.ascontiguousarray(kh8.T).reshape(D, NKC, KC)  # [d, c, m]
        klT = np.ascontiguousarray(kl8.T).reshape(D, NKC, KC)
        # kd[p, c, i, m]: p<64 -> (Kh, Kl)[d=p], p>=64 -> (Kl, Kh)[d=p-64].
        kd = np.empty((KC, NKC, 2, KC), dtype=E4NP)
        kd[0:D, :, 0, :] = khT
        kd[0:D, :, 1, :] = klT
        kd[D:KC, :, 0, :] = klT
        kd[D:KC, :, 1, :] = khT
        # VA[p, c, d] = concat([V, 1])[c*128 + p, d]
        vd = np.ascontiguousarray(
            np.concatenate([V[b], ones], axis=1)
            .reshape(NKC, KC, D + 1)
            .swapaxes(0, 1)
        ).astype(BF16NP)
        in_maps.append({"QD": qd, "KD": kd, "VA": vd})
    return in_maps


def _run(Q, K, V, trace=False, **spmd_kwargs):
    nc = _get_program()
    in_maps = _make_in_maps(Q, K, V)
    res = run_bass_kernel_spmd(
        nc, in_maps, list(range(N_CORES)), trace=trace, **spmd_kwargs
    )
    out = np.empty((B, LQ, D), dtype=np.float32)
    for core in range(N_CORES):
        b, half = core // 2, core % 2
        o = res.results[core]["O"].reshape(KC, 2 * NJ, D + 1)  # [p, j, 65]
        shard = (o[:, :, 0:D] / o[:, :, D : D + 1]).swapaxes(0, 1).reshape(
            LQ_SHARD, D
        )
        out[b, half * LQ_SHARD : (half + 1) * LQ_SHARD, :] = shard
    return out, res


def kernel(Q, K, V):
    out, _ = _run(Q, K, V, trace=False)
    return out


# revision 17
# speedup vs baseline: 1.0261x; 1.0261x over previous
"""Cross-attention kernel for Trainium2 (8 NeuronCores, SPMD data-parallel).

Problem: O = softmax(Q @ K^T) @ V with B=4, Lq=Lk=4096, D=64, fp32 (no
1/sqrt(d) scaling).

Sharding: 8 cores = 4 batches x 2 Lq-halves. Each core handles a
[2048, 64] Q shard against the full [4096, 64] K/V of its batch.
Independent outputs -> no collectives.

Score matmuls use fp8e4 (e4m3) in DoubleRow perf mode (1 out column
per 2.4GHz cycle with 256-deep contraction = 2x the bf16 MAC rate).
Precision is recovered with a 2-level split: Q = Qh + Ql, K = Kh + Kl
(Qh = e4m3(Q), Ql = e4m3(Q - Qh)); one DoubleRow matmul computes all
four cross terms at once:
  out = sum_i lhsT[:,i,:].T @ rhs[:,i,:]   (i = 0,1; contraction 128)
with partition rows 0:64 = d-index "high" terms, 64:128 = "low" terms:
  lhsT[:,0] = [Kh; Kl], lhsT[:,1] = [Kl; Kh] (k on free dim)
  rhs[:,0] = rhs[:,1] = [Qh; Ql]            (q on free dim, duplicated)
=> out = (Kh+Kl) stacked-contraction (Qh+Ql) = Q @ K^T to ~2^-8 rel.
Host-measured end-to-end rel_l2 vs the fp32 reference: ~5.9e-3.

Per-core pipeline (one unit = one k-chunk of 128 keys x 1024 q):
  - ST[k, q]: 4 DoubleRow matmuls of 256 out-cols each (~109ns) into
    two independent single-bank PSUM tiles sta/stb [128, 512] (each
    bufs=3) so the scalar-exp chain (sta) and vector-exp chain (stb)
    decouple and the exp->PSUM-reuse chain never stalls the PE. s=0/2
    start=True marks the 2KB bank pending-zero; s=1/3 start=False
    first-touch writes fresh.
  - P = exp(ST) -> bf16 SBUF: scalar engine does exact table exp on
    sta (q-cols 0:512), vector engine does a Schraudolph bit-trick exp
    (int16(A*s + B) reinterpreted as bf16 ~= e^s) on stb (512:1024).
    pt_a/pt_b come from separate pools (bufs=4 each) so the exp ->
    PV(u) -> pool-reuse chain has 3 units of slack (a shared pool
    halves that and was the main pipeline stall in earlier revisions).
  - Transposed PV, q on output partitions: for each 128-q sub-block j,
    out[q, 0:65] += matmul(lhsT=PT[:, j*128:(j+1)*128], rhs=VA chunk
    [128, 65]) -- 65 bf16 columns per matmul (~30ns back-to-back). VA
    = concat([V, ones], 1): col 64 accumulates the softmax denominator.
  - Each OT PSUM bank ([128, 4*65] = 4 j-groups) starts once, stops
    once. OT banks are pooled per qh (qh=1 reuses qh=0's two banks
    after the explicit mid-loop drain at u=33), which is what frees
    the 2 banks for the deeper sta/stb rotation.
  - 14 junk DoubleRow matmuls run during the input-DMA wait to ramp
    the PE HAM p-state (1.2 -> 2.4 GHz takes a few us of activity) so
    the real loop starts close to warm.
  - Steady state (~765ns/unit) is paced by exp-engine throughput (ACT
    674ns + DVE 689ns per unit, both ~90% busy). Fusing each unit's
    exp into one 1024-col instruction on alternating engines was tried
    and is ~10us WORSE (the PV/ST chains serialize behind the 1118ns
    instruction); the split 512/512 layout is the right shape.
  - Normalization (divide by col 64) happens on host after DMA-out,
    like the host-side transposes/fp8 packing.
"""

import sys

for _p in ("/opt/trn_rl_repo", "/opt/pypackages"):
    if _p not in sys.path:
        sys.path.insert(0, _p)

from contextlib import ExitStack

import ml_dtypes
import numpy as np

import concourse.bacc as bacc
import concourse.mybir as mybir
import concourse.tile as tile
from concourse.bass_utils import run_bass_kernel_spmd

# Problem constants (hardcoded per contract).
B, LQ, LK, D = 4, 4096, 4096, 64
N_CORES = 8
LQ_SHARD = LQ * B // N_CORES  # 2048
KC = 128  # k-chunk (PV contraction tile)
NKC = LK // KC  # 32
QB = 1024  # q extent per unit
NQB = LQ_SHARD // QB  # 2
NU = NQB * NKC  # 64 units
SW = 256  # out columns per DoubleRow score matmul (rhs free = 512 = max)
HB = 512  # half-unit q extent (one PSUM bank / one exp engine)
NJ = QB // 128  # q sub-blocks per unit (8)
N_WARM = 14  # junk matmuls to ramp the PE p-state during the DMA wait

F32 = mybir.dt.float32
BF16 = mybir.dt.bfloat16
I16 = mybir.dt.int16
E4 = mybir.dt.float8e4

BF16NP = ml_dtypes.bfloat16
E4NP = ml_dtypes.float8_e4m3

# Schraudolph constants for bf16: int16(A*s + B) bits viewed as bf16 ~ e^s.
SCH_A = float(128.0 / np.log(2.0))  # 184.664...
SCH_C = 8.0  # sawtooth centering shift
SCH_B = 128.0 * 127.0 - SCH_C + 0.5  # +0.5: float->int16 cast truncates


def _build_program():
    nc = bacc.Bacc(
        "TRN2",
        target_bir_lowering=False,
        debug=False,
        num_devices=N_CORES,
    )
    # [p, qh, i, col]: p<64 -> Qh[qh*1024+col, p], p>=64 -> Ql; i duplicated.
    q_d = nc.declare_dram_parameter("QD", [KC, NQB, 2, QB], E4, isOutput=False)
    k_d = nc.declare_dram_parameter("KD", [KC, NKC, 2, KC], E4, isOutput=False)
    va_d = nc.declare_dram_parameter("VA", [KC, NKC, D + 1], BF16, isOutput=False)
    # O[p, j*65 + d]: q sub-block j (q = j*128 + p), d in 0:64 out, 64 = den.
    o_d = nc.declare_dram_parameter("O", [KC, 2 * NJ * (D + 1)], F32, isOutput=True)

    with tile.TileContext(nc) as tc, ExitStack() as ctx:
        singles = ctx.enter_context(tc.tile_pool(name="singles", bufs=1))
        sta_pool = ctx.enter_context(tc.tile_pool(name="sta", bufs=3, space="PSUM"))
        stb_pool = ctx.enter_context(tc.tile_pool(name="stb", bufs=3, space="PSUM"))
        ot_pool = ctx.enter_context(tc.tile_pool(name="ot", bufs=1, space="PSUM"))
        pta_pool = ctx.enter_context(tc.tile_pool(name="pta", bufs=4))
        ptb_pool = ctx.enter_context(tc.tile_pool(name="ptb", bufs=4))
        ob_pool = ctx.enter_context(tc.tile_pool(name="ob", bufs=1))

        # Preload the exp activation table while input DMAs run.
        warm = singles.tile([1, 2], F32)
        nc.vector.memset(warm[:, :], 0.0)
        nc.scalar.activation(
            out=warm[:, :], in_=warm[:, :],
            func=mybir.ActivationFunctionType.Exp,
        )

        qsb = singles.tile([KC, NQB, 2, QB], E4, name="qsb")
        ksb = singles.tile([KC, NKC, 2, KC], E4, name="ksb")
        va = singles.tile([KC, NKC, D + 1], BF16, name="va")

        # Input DMAs, earliest-needed-first; small lead pieces so the
        # first score matmul is gated on ~320KB, not the full 2MB.
        nc.sync.dma_start(out=ksb[:, 0:2], in_=k_d[:, 0:2])
        nc.sync.dma_start(out=qsb[:, 0], in_=q_d[:, 0])
        nc.sync.dma_start(out=va[:, 0:2, :], in_=va_d[:, 0:2, :])
        nc.sync.dma_start(out=ksb[:, 2:8], in_=k_d[:, 2:8])
        nc.sync.dma_start(out=va[:, 2:8, :], in_=va_d[:, 2:8, :])
        nc.sync.dma_start(out=ksb[:, 8:NKC], in_=k_d[:, 8:NKC])
        nc.sync.dma_start(out=va[:, 8:NKC, :], in_=va_d[:, 8:NKC, :])
        nc.sync.dma_start(out=qsb[:, 1], in_=q_d[:, 1])

        # PE p-state warmup: junk DoubleRow matmuls on memset SBUF while
        # the input DMAs land (HAM ramps 1.2 -> 2.4 GHz only after a few
        # us of PE activity; these absorb most of that at otherwise-idle
        # time). They cycle the sta/stb pools ahead of the real loop.
        junk = singles.tile([KC, 2, SW], E4, name="junk")
        nc.gpsimd.memset(junk[:, :, :], 0.0)
        for w in range(N_WARM):
            if w % 2 == 0:
                wst = sta_pool.tile([KC, HB], F32, name=f"wst{w}", tag="sta")
            else:
                wst = stb_pool.tile([KC, HB], F32, name=f"wst{w}", tag="stb")
            nc.tensor.matmul(
                out=wst[:, 0:SW],
                lhsT=junk[:, :, 0:KC],
                rhs=junk[:, :, :],
                start=True,
                stop=True,
                perf_mode=mybir.MatmulPerfMode.DoubleRow,
                skip_group_check=True,
            )

        # One PSUM bank per OT tile (4 j-groups of 65 cols each), pooled
        # per qh: qh=1 reuses qh=0's two banks after the mid-loop drain,
        # freeing 2 banks for the deeper sta/stb rotation.
        ots = {}

        # Software-pipelined main loop: unit u = (qh, c) = (u // NKC, u % NKC).
        pts = [None] * NU

        def emit_scores_exp(u):
            qh, c = divmod(u, NKC)
            sta = sta_pool.tile([KC, HB], F32, tag="sta")
            stb = stb_pool.tile([KC, HB], F32, tag="stb")
            for s in range(4):
                tgt = sta if s < 2 else stb
                nc.tensor.matmul(
                    out=tgt[:, (s % 2) * SW : (s % 2 + 1) * SW],
                    lhsT=ksb[:, c, :, :],
                    rhs=qsb[:, qh, :, s * SW : (s + 1) * SW],
                    start=(s % 2 == 0),
                    stop=(s % 2 == 1),
                    perf_mode=mybir.MatmulPerfMode.DoubleRow,
                    skip_group_check=True,
                )
            pt_a = pta_pool.tile([KC, HB], BF16, tag="pta")
            pt_b = ptb_pool.tile([KC, HB], BF16, tag="ptb")
            nc.scalar.activation(
                out=pt_a[:, :],
                in_=sta[:, :],
                func=mybir.ActivationFunctionType.Exp,
            )
            nc.vector.tensor_scalar(
                pt_b[:, :].bitcast(I16),
                stb[:, :],
                SCH_A,
                SCH_B,
                mybir.AluOpType.mult,
                mybir.AluOpType.add,
            )
            pts[u] = (pt_a, pt_b)

        def emit_pv(u):
            qh, c = divmod(u, NKC)
            if c == 0:
                ots[qh] = [
                    ot_pool.tile(
                        [KC, 4 * (D + 1)], F32, name=f"ot{qh}{h}", tag=f"ot{h}"
                    )
                    for h in range(2)
                ]
            pt_a, pt_b = pts[u]
            for j in range(NJ):
                if j < 4:
                    lhsT = pt_a[:, j * KC : (j + 1) * KC]
                else:
                    lhsT = pt_b[:, (j - 4) * KC : (j - 3) * KC]
                h, g = j // 4, j % 4
                nc.tensor.matmul(
                    out=ots[qh][h][:, g * (D + 1) : (g + 1) * (D + 1)],
                    lhsT=lhsT,
                    rhs=va[:, c, :],
                    start=(c == 0 and g == 0),
                    stop=(c == NKC - 1 and g == 3),
                    skip_group_check=True,
                )

        # Output drain (normalization on host): PSUM->SBUF copies split
        # across the scalar and vector engines, then one DMA per qh
        # half. qh=0 drains mid-loop (its OT banks finish at u=31).
        W = 4 * (D + 1)
        ob = ob_pool.tile([KC, 4, W], F32, name="ob")

        def drain(qh):
            for h in range(2):
                t = qh * 2 + h
                if h == 0:
                    nc.scalar.activation(
                        out=ob[:, t, :], in_=ots[qh][h][:, :],
                        func=mybir.ActivationFunctionType.Copy,
                    )
                else:
                    nc.vector.tensor_copy(ob[:, t, :], ots[qh][h][:, :])
                eng = nc.sync if h == 0 else nc.gpsimd
                eng.dma_start(
                    out=o_d[:, t * W : (t + 1) * W],
                    in_=ob[:, t, :],
                )

        emit_scores_exp(0)
        emit_scores_exp(1)
        for u in range(2, NU):
            emit_scores_exp(u)
            emit_pv(u - 2)
            if u == 33:
                drain(0)
        emit_pv(NU - 2)
        emit_pv(NU - 1)
        drain(1)

    nc.finalize()
    return nc


_PROGRAM_CACHE = {}


def _get_program():
    if "nc" not in _PROGRAM_CACHE:
        _PROGRAM_CACHE["nc"] = _build_program()
    return _PROGRAM_CACHE["nc"]


def _make_in_maps(Q, K, V):
    Q = np.asarray(Q, dtype=np.float32)
    K = np.asarray(K, dtype=np.float32)
    V = np.asarray(V, dtype=np.float32)
    in_maps = []
    ones = np.ones((LK, 1), dtype=np.float32)
    for core in range(N_CORES):
        b, half = core // 2, core % 2
        q = Q[b, half * LQ_SHARD : (half + 1) * LQ_SHARD, :]  # [2048, 64]
        qh8 = q.astype(E4NP)
        ql8 = (q - qh8.astype(np.float32)).astype(E4NP)
        # qd[p, qh, i, col]: p<64 -> Qh[qh*1024+col, p]; p>=64 -> Ql.
        qd = np.empty((KC, NQB, 2, QB), dtype=E4NP)
        qhT = qh8.T.reshape(D, NQB, QB)
        qlT = ql8.T.reshape(D, NQB, QB)
        qd[0:D, :, 0, :] = qhT
        qd[0:D, :, 1, :] = qhT
        qd[D:KC, :, 0, :] = qlT
        qd[D:KC, :, 1, :] = qlT
        k = K[b]  # [4096, 64]
        kh8 = k.astype(E4NP)
        kl8 = (k - kh8.astype(np.float32)).astype(E4NP)
        khT = np.ascontiguousarray(kh8.T).reshape(D, NKC, KC)  # [d, c, m]
        klT = np.ascontiguousarray(kl8.T).reshape(D, NKC, KC)
        # kd[p, c, i, m]: p<64 -> (Kh, Kl)[d=p], p>=64 -> (Kl, Kh)[d=p-64].
        kd = np.empty((KC, NKC, 2, KC), dtype=E4NP)
        kd[0:D, :, 0, :] = khT
        kd[0:D, :, 1, :] = klT
        kd[D:KC, :, 0, :] = klT
        kd[D:KC, :, 1, :] = khT
        # VA[p, c, d] = concat([V, 1])[c*128 + p, d]
        vd = np.ascontiguousarray(
            np.concatenate([V[b], ones], axis=1)
            .reshape(NKC, KC, D + 1)
            .swapaxes(0, 1)
        ).astype(BF16NP)
        in_maps.append({"QD": qd, "KD": kd, "VA": vd})
    return in_maps


def _run(Q, K, V, trace=False, **spmd_kwargs):
    nc = _get_program()
    in_maps = _make_in_maps(Q, K, V)
    res = run_bass_kernel_spmd(
        nc, in_maps, list(range(N_CORES)), trace=trace, **spmd_kwargs
    )
    out = np.empty((B, LQ, D), dtype=np.float32)
    for core in range(N_CORES):
        b, half = core // 2, core % 2
        o = res.results[core]["O"].reshape(KC, 2 * NJ, D + 1)  # [p, j, 65]
        shard = (o[:, :, 0:D] / o[:, :, D : D + 1]).swapaxes(0, 1).reshape(
            LQ_SHARD, D
        )
        out[b, half * LQ_SHARD : (half + 1) * LQ_SHARD, :] = shard
    return out, res


def kernel(Q, K, V):
    out, _ = _run(Q, K, V, trace=False)
    return out
